# revision 29
# baseline (speedup 1.0000x reference)
"""Trainium2 Bass kernel for nn_BlockedMLP (dense_mlp, 8 cores).

Strategy:
  - 8-way data parallel over the batch (B=2048 -> 256 rows/core), weights
    replicated. No collectives.
  - The BSR fc2 (50% block density, 32x32 blocks) is scattered into a dense
    [H, H] matrix on the host: on the PE array a matmul costs N streamed
    columns regardless of contraction K, so 32x32 sparse blocks waste ~4x
    throughput vs dense 128x128 tiles and the block gather costs more than
    the 2x FLOP saving.
  - Feature-major ("transposed") layout throughout: activations live in SBUF
    as [feature_partition, batch_free]; weights are the stationary matmul
    operand, activations stream. Host pre-transposes x and the weights, so
    the device kernel needs no transposes at all.
  - bf16 inputs/weights (host cast) with fp32 PSUM accumulation: 1 cycle/row
    on the PE (fp32 is 4) and half the HBM traffic.
  - v2 schedule: every weight tile gets a DEDICATED SBUF slot (the whole
    16.8 MB stream fits: 128 KB/partition of the 208 KB budget), so all
    weight DMAs issue unconditionally at kernel start and the two HWDGE
    queues stream flat-out with zero slot-reuse waits (v1 lost ~2.7 us to a
    16-slot ring stall at the fc1->fc2 boundary, plus a HAM down-clock it
    triggered).  fc1/fc2 run as 4-j-tile waves alternating between two PSUM
    bank groups, so a wave's matmuls never wait on the previous wave's
    epilogues.  Epilogues run on Vector/GpSimd (alternating), keeping
    Scalar/Sync free for DMA issue and dropping the Scalar ACT-table load.
    The weight stream leads with small (128 KB) tiles so the first real
    matmul starts ~5 us earlier; PE warmup matmuls cover the DMA-queue
    ramp to keep the HAM clock ramping toward 2.4 GHz.  fc3 runs j-outer
    with per-j epilogue+store so only the last tile's epilogue is tail.
"""

import numpy as np
import ml_dtypes

try:
    import concourse.bass as bass  # noqa: F401
except ImportError:
    import sys

    for _p in ("/opt/trn_rl_repo", "/root/.axon_site/_ro/trn_rl_repo"):
        if _p not in sys.path:
            sys.path.insert(0, _p)

import concourse.bacc as bacc
import concourse.bass as bass
import concourse.mybir as mybir
import concourse.tile as tile
from concourse import bass_utils

LIGHT_TAIL = True  # replace Tile's heavy end-of-kernel barrier with a minimal one
FAST_CONST = True  # route Bass-init const-AP memsets to VectorE (GpSimd is ~8x slower)

B, IN, H, OUT, BS = 2048, 1024, 2048, 1024, 32
NCORES = 8
BSH = B // NCORES  # 256 batch rows per core
P = 128

F32 = mybir.dt.float32
ADD = mybir.AluOpType.add
MAX = mybir.AluOpType.max

KT1, KT2, KT3 = IN // P, H // P, H // P  # 8, 16, 16
W1J, W2J, W3J = H // P, H // P, OUT // P  # 16, 16, 8 j-tiles total
WAVE_J = 4  # j-tiles per wave for fc1/fc2 (PSUM bank-group double buffering)
NW1, NW2 = W1J // WAVE_J, W2J // WAVE_J  # 4, 4

# DMA plan: ("w", layer, wave, k0, merge, width) weight tiles and
# ("x", klo, khi) activation chunks, in earliest-deadline order. Entries
# alternate between the two HWDGE queues; the early phase is bandwidth-
# bound (~150-200 GB/s aggregate until ~14us), so everything is fine-
# grained (64-256 KB) and ordered strictly by when the PE needs it.
def _dma_plan():
    plan = []
    # fc1 wave0 weights interleaved with the x chunks they run against
    # (x[0:2] rides the GpSimd SWDGE queue as extra early bandwidth).
    xq = {2: ("x", 2, 4), 5: ("x", 4, 6), 8: ("x", 6, 8)}
    kk = 0
    for k in range(KT1):
        plan.append(("w", "l1", 0, k, 1, WAVE_J * P))
        kk += 1
        if kk in xq:
            plan.append(xq[kk])
    # fc1 waves 1-3: per-k tiles for wave1 (still ramp-limited), pairs after.
    for wv in range(1, NW1):
        merges = [1] * 8 if wv == 1 else [2, 2, 2, 2]
        k0 = 0
        for m in merges:
            plan.append(("w", "l1", wv, k0, m, WAVE_J * P))
            k0 += m
        assert k0 == KT1
    for wv in range(NW2):
        k0 = 0
        # waves 0-1 finer-grained: their tiles arrive while the stream is
        # still catching up from the ramp, and a fine tile bounds each PE
        # wait well below the ~1.5us HAM down-clock trigger; later waves
        # have slack.
        merges = {0: [2] * 8, 1: [2, 2, 4, 4, 4]}.get(wv, [4, 4, 4, 4])
        for m in merges:
            plan.append(("w", "l2", wv, k0, m, WAVE_J * P))
            k0 += m
        assert k0 == KT2
    k0 = 0
    for m in [2] * 8:
        plan.append(("w", "l3", 0, k0, m, W3J * P))
        k0 += m
    assert k0 == KT3
    return plan


DMA_PLAN = _dma_plan()
WSEQ_COLS = sum(e[4] * e[5] for e in DMA_PLAN if e[0] == "w")  # 65536

_CACHE = {}


def _emit(tc, DT):
    nc = tc.nc

    xT = nc.dram_tensor("xT", [P, KT1, BSH], DT, kind="ExternalInput").ap()
    wseq = nc.dram_tensor("wseq", [P, WSEQ_COLS], DT, kind="ExternalInput").ap()
    bc = nc.dram_tensor("bc", [P, W1J + W2J + W3J], F32, kind="ExternalInput").ap()
    # Output in bf16: halves the final-store latency (the tail's longest
    # chain) and the store traffic; costs ~0.2% extra rel err (budget 2%).
    outT = nc.dram_tensor("outT", [W3J, P, BSH], DT, kind="ExternalOutput").ap()

    from contextlib import ExitStack

    with ExitStack() as ctx:
        wp = ctx.enter_context(tc.tile_pool(name="wpool", bufs=1))
        act = ctx.enter_context(tc.tile_pool(name="act", bufs=1))
        pp = ctx.enter_context(tc.tile_pool(name="ps", bufs=1, space="PSUM"))
        iop = ctx.enter_context(tc.tile_pool(name="io", bufs=1))

        # x[0:2] + biases ride the GpSimd SWDGE queue (slow but extra early
        # bandwidth); the rest of x is interleaved into the HWDGE streams.
        xt = iop.tile([P, KT1, BSH], DT, tag="x", name="xt")
        nc.gpsimd.dma_start(xt[:, 0:2, :], xT[:, 0:2, :])
        xts = [xt[:, k, :] for k in range(KT1)]
        bs = iop.tile([P, W1J + W2J + W3J], F32, tag="bs", name="bs")
        nc.gpsimd.dma_start(bs[:], bc[:])
        b1c = lambda j: bs[:, j : j + 1]
        b2c = lambda j: bs[:, W1J + j : W1J + j + 1]
        b3c = lambda j: bs[:, W1J + W2J + j : W1J + W2J + j + 1]

        # All weight DMAs issue up front into dedicated slots: the HWDGE
        # queues then stream the full 16.8 MB back-to-back with no waits.
        dmaq = [nc.sync, nc.scalar]
        wslice = {}  # (layer, wave, k) -> (tile, col_base)
        off = 0
        for d, e in enumerate(DMA_PLAN):
            if e[0] == "x":
                _, lo, hi = e
                dmaq[d % 2].dma_start(xt[:, lo:hi, :], xT[:, lo:hi, :])
                continue
            _, layer, wv, k0, merge, width = e
            w = wp.tile([P, merge * width], DT, tag=f"w{d}", name=f"w_{layer}x{wv}k{k0}")
            dmaq[d % 2].dma_start(w[:], wseq[:, off : off + merge * width])
            off += merge * width
            for kk in range(merge):
                wslice[(layer, wv, k0 + kk)] = (w, kk * width)

        # PE warmup: real matmuls can't start until the first weight tile
        # lands (~9.5us: DMA queue ramp); dummy matmuls on zeroed SBUF keep
        # the HAM clock ramping toward 2.4 GHz meanwhile.
        warm_rhs = iop.tile([P, BSH], mybir.dt.bfloat16, tag="warm_rhs", name="warm_rhs")
        nc.vector.memset(warm_rhs[:], 0.0)
        warm_ps = pp.tile([P, BSH], F32, tag="pA0", name="warm_ps")
        # ~21 warmups put the first real matmul at ~10.8us, matching the DMA
        # delivery curve (~200-390 GB/s from ~8.5us, slower on contended
        # devices) so the weight stream rarely starves the PE (a >1.5us PE
        # idle triggers a HAM down-clock to half speed; that 3.4-10us
        # straggler penalty dominates the max-core metric, so margin here
        # is worth ~0.6us of fixed cost).
        for i in range(21):
            nc.tensor.matmul(
                warm_ps[:], warm_rhs[:, 0:P], warm_rhs[:], start=True, stop=True
            )

        # All epilogues on Vector (GpSimd can't read PSUM; Scalar would need
        # the ACT-table load and is busy issuing weight DMAs). With PSUM
        # bank-group double buffering the epilogue latency never gates the
        # matmul stream, and Vector is otherwise idle (~19us work total).
        def epilogue(o, ps_ap, bias_ap, relu):
            if relu:
                nc.vector.tensor_scalar(o, ps_ap, bias_ap, 0.0, ADD, MAX)
            else:
                nc.vector.tensor_scalar_add(o, ps_ap, bias_ap)

        bank = {
            "A": ["pA0", "pA1", "pA2", "pA3"],
            "B": ["pB0", "pB1", "pB2", "pB3"],
        }

        def wave(layer, wv, kt, rhs, bias_col, out_dt, grp):
            ps = [
                pp.tile([P, BSH], F32, tag=bank[grp][i], name=f"{layer}w{wv}ps{i}")
                for i in range(WAVE_J)
            ]
            for k in range(kt):
                w, base = wslice[(layer, wv, k)]
                for j in range(WAVE_J):
                    nc.tensor.matmul(
                        ps[j][:],
                        w[:, base + j * P : base + (j + 1) * P],
                        rhs[k],
                        start=(k == 0),
                        stop=(k == kt - 1),
                    )
            outs = []
            for j in range(WAVE_J):
                o = act.tile(
                    [P, BSH], out_dt, tag=f"{layer}w{wv}o{j}", name=f"{layer}w{wv}o{j}"
                )
                epilogue(o[:], ps[j][:], bias_col(wv * WAVE_J + j), True)
                outs.append(o[:])
            return outs

        hts = []
        for wv in range(NW1):
            hts += wave("l1", wv, KT1, xts, b1c, DT, "AB"[wv % 2])
        h2s = []
        for wv in range(NW2):
            h2s += wave("l2", wv, KT2, hts, b2c, DT, "AB"[wv % 2])

        # fc3 j-outer: each output tile's epilogue + store overlaps the next
        # tile's matmuls; only the last tile's epilogue+store is tail latency.
        tags8 = bank["A"] + bank["B"]
        for j in range(W3J):
            psj = pp.tile([P, BSH], F32, tag=tags8[j], name=f"l3ps{j}")
            for k in range(KT3):
                w, base = wslice[("l3", 0, k)]
                nc.tensor.matmul(
                    psj[:],
                    w[:, base + j * P : base + (j + 1) * P],
                    h2s[k],
                    start=(k == 0),
                    stop=(k == KT3 - 1),
                )
            o = act.tile([P, BSH], DT, tag=f"l3o{j}", name=f"l3o{j}")
            # Split epilogue+store across both queues: halves transfer in
            # parallel, and for the last tile the first store can issue
            # while the second epilogue half still runs — this chain is the
            # kernel's tail latency.
            h = BSH // 2
            epilogue(o[:, 0:h], psj[:, 0:h], b3c(j), False)
            nc.sync.dma_start(outT[j][:, 0:h], o[:, 0:h])
            epilogue(o[:, h:BSH], psj[:, h:BSH], b3c(j), False)
            nc.scalar.dma_start(outT[j][:, h:BSH], o[:, h:BSH])


class _LightTailTileContext(tile.TileContext):
    """TileContext with a minimal end-of-kernel sequence.

    Tile's default tail (drain + full all-engine barrier + DMA/semaphore
    reset + second barrier) costs ~8-10us on HW. For a single-TileContext
    kernel the correctness requirement at the end is just: all engines done
    and all output DMAs complete before the NEFF signals completion (the
    walrus-generated per-engine teardown follows anyway).
    """

    def _drain_and_barrier(self, tick_clock, wait_clock):
        if not hasattr(self.nc, "_tile_sem_poison_stack"):
            return super()._drain_and_barrier(tick_clock, wait_clock)
        from concourse.vector_clock import ScopedClock

        drain_inst = self.nc.sync.drain()
        wait_clock.add_sem_waits(
            drain_inst.ins, ScopedClock({None: tick_clock.global_clock})
        )
        self.nc.all_engine_barrier(sem_only=True)
        assert self.sems is not None
        popped = self.nc._tile_sem_poison_stack.pop()
        assert popped is self._sem_poison

def _build(dt_name):
    if dt_name in _CACHE:
        return _CACHE[dt_name]
    DT = {"bf16": mybir.dt.bfloat16, "f32": F32}[dt_name]

    patches = []
    if FAST_CONST:
        try:
            import concourse.bass as cbass

            # During Bass construction only, reroute GpSimd memsets (the
            # framework's const-AP init) to the much faster VectorE: they
            # gate the initial all-engine barrier.
            gps_cls = cbass.BassGpSimd

            def memset_shim(self, ap, constant):
                return self.bass.vector.memset(ap, constant)

            had = "memset" in vars(gps_cls)
            orig = vars(gps_cls).get("memset")
            gps_cls.memset = memset_shim
            patches.append((gps_cls, "memset", had, orig))
            # The barrier after const-AP init protects readers of the const
            # tiles; this kernel never reads them, so skip it.
            bar_orig = cbass.Bass.all_engine_barrier

            def bar_shim(self, *, sem_only=False):
                return None

            cbass.Bass.all_engine_barrier = bar_shim
            patches.append((cbass.Bass, "all_engine_barrier", True, bar_orig))
        except AttributeError:
            pass

    try:
        nc = bacc.Bacc(
            "TRN2",
            target_bir_lowering=False,
            debug=False,
            enable_asserts=False,
            num_devices=NCORES,
        )
    finally:
        for klass, attr, had, orig in patches:
            if had:
                setattr(klass, attr, orig)
            else:
                delattr(klass, attr)

    tc_cls = _LightTailTileContext if LIGHT_TAIL else tile.TileContext
    with tc_cls(nc) as tc:
        _emit(tc, DT)
    nc.compile()
    _CACHE[dt_name] = nc
    return nc


def _np_dt(dt_name):
    return mybir.dt.np({"bf16": mybir.dt.bfloat16, "f32": F32}[dt_name])


def _host_prep(x, W1, b1, crow_indices, col_indices, values, b2, W3, b3, npdt):
    rb = crow_indices.shape[0] - 1
    nnz, bs, _ = values.shape
    cb = H // bs
    # Scatter BSR into dense W2 [H, H].
    blocks = np.zeros((rb, cb, bs, bs), np.float32)
    row_ids = (
        np.searchsorted(crow_indices, np.arange(nnz, dtype=np.int64), side="right") - 1
    )
    blocks[row_ids, col_indices] = values
    W2 = blocks.transpose(0, 2, 1, 3).reshape(H, H)

    WT = {
        "l1": np.ascontiguousarray(W1.T).astype(npdt),  # [IN, H]
        "l2": np.ascontiguousarray(W2.T).astype(npdt),  # [H, H]
        "l3": np.ascontiguousarray(W3.T).astype(npdt),  # [H, OUT]
    }
    # Pack the streamed weight sequence: one contiguous [P, merge*width]
    # block per DMA instruction, in consumption order.
    blocks_out = []
    for e in DMA_PLAN:
        if e[0] == "x":
            continue
        _, layer, wv, k0, merge, width = e
        w = WT[layer]
        jbase = wv * width
        blocks_out.append(
            np.concatenate(
                [
                    w[(k0 + kk) * P : (k0 + kk + 1) * P, jbase : jbase + width]
                    for kk in range(merge)
                ],
                axis=1,
            )
        )
    wseq = np.ascontiguousarray(np.concatenate(blocks_out, axis=1))
    assert wseq.shape == (P, WSEQ_COLS)

    bc = np.ascontiguousarray(
        np.concatenate(
            [
                b1.reshape(W1J, P).T,
                b2.reshape(W2J, P).T,
                b3.reshape(W3J, P).T,
            ],
            axis=1,
        ).astype(np.float32)
    )
    # x -> per-core transposed shards [P, KT1, BSH] in natural k order.
    xT_all = np.ascontiguousarray(x.T.astype(npdt))  # [IN, B]
    shards = [
        np.ascontiguousarray(
            xT_all[:, c * BSH : (c + 1) * BSH].reshape(KT1, P, BSH).transpose(1, 0, 2)
        )
        for c in range(NCORES)
    ]
    shared = dict(wseq=wseq, bc=bc)
    return [dict(shared, xT=shards[c]) for c in range(NCORES)]


def kernel(x, W1, b1, crow_indices, col_indices, values, b2, W3, b3, _dt="bf16"):
    nc = _build(_dt)
    in_maps = _host_prep(
        np.asarray(x, np.float32),
        np.asarray(W1, np.float32),
        np.asarray(b1, np.float32),
        np.asarray(crow_indices),
        np.asarray(col_indices),
        np.asarray(values, np.float32),
        np.asarray(b2, np.float32),
        np.asarray(W3, np.float32),
        np.asarray(b3, np.float32),
        _np_dt(_dt),
    )
    res = bass_utils.run_bass_kernel_spmd(nc, in_maps, core_ids=list(range(NCORES)))
    out = np.concatenate(
        [res.results[c]["outT"].reshape(OUT, BSH).T for c in range(NCORES)], axis=0
    )
    return np.ascontiguousarray(out.astype(np.float32))


# revision 30
# speedup vs baseline: 1.0500x; 1.0500x over previous
"""Trainium2 Bass kernel for nn_BlockedMLP (dense_mlp, 8 cores).

Strategy:
  - 8-way data parallel over the batch (B=2048 -> 256 rows/core), weights
    replicated. No collectives.
  - The BSR fc2 (50% block density, 32x32 blocks) is scattered into a dense
    [H, H] matrix on the host: on the PE array a matmul costs N streamed
    columns regardless of contraction K, so 32x32 sparse blocks waste ~4x
    throughput vs dense 128x128 tiles and the block gather costs more than
    the 2x FLOP saving.
  - Feature-major ("transposed") layout throughout: activations live in SBUF
    as [feature_partition, batch_free]; weights are the stationary matmul
    operand, activations stream. Host pre-transposes x and the weights, so
    the device kernel needs no transposes at all.
  - bf16 inputs/weights (host cast) with fp32 PSUM accumulation: 1 cycle/row
    on the PE (fp32 is 4) and half the HBM traffic.
  - v2 schedule: every weight tile gets a DEDICATED SBUF slot (the whole
    16.8 MB stream fits: 128 KB/partition of the 208 KB budget), so all
    weight DMAs issue unconditionally at kernel start and the two HWDGE
    queues stream flat-out with zero slot-reuse waits (v1 lost ~2.7 us to a
    16-slot ring stall at the fc1->fc2 boundary, plus a HAM down-clock it
    triggered).  fc1/fc2 run as 4-j-tile waves alternating between two PSUM
    bank groups, so a wave's matmuls never wait on the previous wave's
    epilogues.  Epilogues run on Vector/GpSimd (alternating), keeping
    Scalar/Sync free for DMA issue and dropping the Scalar ACT-table load.
    The weight stream leads with small (128 KB) tiles so the first real
    matmul starts ~5 us earlier; PE warmup matmuls cover the DMA-queue
    ramp to keep the HAM clock ramping toward 2.4 GHz.  fc3 runs j-outer
    with per-j epilogue+store so only the last tile's epilogue is tail.
"""

import numpy as np
import ml_dtypes

try:
    import concourse.bass as bass  # noqa: F401
except ImportError:
    import sys

    for _p in ("/opt/trn_rl_repo", "/root/.axon_site/_ro/trn_rl_repo"):
        if _p not in sys.path:
            sys.path.insert(0, _p)

import concourse.bacc as bacc
import concourse.bass as bass
import concourse.mybir as mybir
import concourse.tile as tile
from concourse import bass_utils

LIGHT_TAIL = True  # replace Tile's heavy end-of-kernel barrier with a minimal one
FAST_CONST = True  # route Bass-init const-AP memsets to VectorE (GpSimd is ~8x slower)

B, IN, H, OUT, BS = 2048, 1024, 2048, 1024, 32
NCORES = 8
BSH = B // NCORES  # 256 batch rows per core
P = 128

F32 = mybir.dt.float32
ADD = mybir.AluOpType.add
MAX = mybir.AluOpType.max

KT1, KT2, KT3 = IN // P, H // P, H // P  # 8, 16, 16
W1J, W2J, W3J = H // P, H // P, OUT // P  # 16, 16, 8 j-tiles total
WAVE_J = 4  # j-tiles per wave for fc1/fc2 (PSUM bank-group double buffering)
NW1, NW2 = W1J // WAVE_J, W2J // WAVE_J  # 4, 4

# DMA plan: ("w", layer, wave, k0, merge, width) weight tiles and
# ("x", klo, khi) activation chunks, in earliest-deadline order. Entries
# alternate between the two HWDGE queues; the early phase is bandwidth-
# bound (~150-200 GB/s aggregate until ~14us), so everything is fine-
# grained (64-256 KB) and ordered strictly by when the PE needs it.
def _dma_plan():
    plan = []
    # fc1 wave0 weights interleaved with the x chunks they run against
    # (x[0:2] rides the GpSimd SWDGE queue as extra early bandwidth).
    xq = {2: ("x", 2, 4), 5: ("x", 4, 6), 8: ("x", 6, 8)}
    kk = 0
    for k in range(KT1):
        plan.append(("w", "l1", 0, k, 1, WAVE_J * P))
        kk += 1
        if kk in xq:
            plan.append(xq[kk])
    # fc1 waves 1-3: per-k tiles for wave1 (still ramp-limited), pairs after.
    for wv in range(1, NW1):
        merges = [1] * 8 if wv == 1 else [2, 2, 2, 2]
        k0 = 0
        for m in merges:
            plan.append(("w", "l1", wv, k0, m, WAVE_J * P))
            k0 += m
        assert k0 == KT1
    for wv in range(NW2):
        k0 = 0
        # waves 0-1 finer-grained: their tiles arrive while the stream is
        # still catching up from the ramp, and a fine tile bounds each PE
        # wait well below the ~1.5us HAM down-clock trigger; later waves
        # have slack.
        merges = {0: [2] * 8, 1: [2, 2, 4, 4, 4]}.get(wv, [4, 4, 4, 4])
        for m in merges:
            plan.append(("w", "l2", wv, k0, m, WAVE_J * P))
            k0 += m
        assert k0 == KT2
    k0 = 0
    for m in [2] * 8:
        plan.append(("w", "l3", 0, k0, m, W3J * P))
        k0 += m
    assert k0 == KT3
    return plan


DMA_PLAN = _dma_plan()
WSEQ_COLS = sum(e[4] * e[5] for e in DMA_PLAN if e[0] == "w")  # 65536

_CACHE = {}


def _emit(tc, DT):
    nc = tc.nc

    xT = nc.dram_tensor("xT", [P, KT1, BSH], DT, kind="ExternalInput").ap()
    wseq = nc.dram_tensor("wseq", [P, WSEQ_COLS], DT, kind="ExternalInput").ap()
    bc = nc.dram_tensor("bc", [P, W1J + W2J + W3J], F32, kind="ExternalInput").ap()
    # Output in bf16: halves the final-store latency (the tail's longest
    # chain) and the store traffic; costs ~0.2% extra rel err (budget 2%).
    outT = nc.dram_tensor("outT", [W3J, P, BSH], DT, kind="ExternalOutput").ap()

    from contextlib import ExitStack

    with ExitStack() as ctx:
        wp = ctx.enter_context(tc.tile_pool(name="wpool", bufs=1))
        act = ctx.enter_context(tc.tile_pool(name="act", bufs=1))
        pp = ctx.enter_context(tc.tile_pool(name="ps", bufs=1, space="PSUM"))
        iop = ctx.enter_context(tc.tile_pool(name="io", bufs=1))

        # x[0:2] + biases ride the GpSimd SWDGE queue (slow but extra early
        # bandwidth); the rest of x is interleaved into the HWDGE streams.
        xt = iop.tile([P, KT1, BSH], DT, tag="x", name="xt")
        nc.gpsimd.dma_start(xt[:, 0:2, :], xT[:, 0:2, :])
        xts = [xt[:, k, :] for k in range(KT1)]
        bs = iop.tile([P, W1J + W2J + W3J], F32, tag="bs", name="bs")
        nc.gpsimd.dma_start(bs[:], bc[:])
        b1c = lambda j: bs[:, j : j + 1]
        b2c = lambda j: bs[:, W1J + j : W1J + j + 1]
        b3c = lambda j: bs[:, W1J + W2J + j : W1J + W2J + j + 1]

        # All weight DMAs issue up front into dedicated slots: the HWDGE
        # queues then stream the full 16.8 MB back-to-back with no waits.
        dmaq = [nc.sync, nc.scalar]
        wslice = {}  # (layer, wave, k) -> (tile, col_base)
        off = 0
        for d, e in enumerate(DMA_PLAN):
            if e[0] == "x":
                _, lo, hi = e
                dmaq[d % 2].dma_start(xt[:, lo:hi, :], xT[:, lo:hi, :])
                continue
            _, layer, wv, k0, merge, width = e
            w = wp.tile([P, merge * width], DT, tag=f"w{d}", name=f"w_{layer}x{wv}k{k0}")
            dmaq[d % 2].dma_start(w[:], wseq[:, off : off + merge * width])
            off += merge * width
            for kk in range(merge):
                wslice[(layer, wv, k0 + kk)] = (w, kk * width)

        # PE warmup: real matmuls can't start until the first weight tile
        # lands (~9.5us: DMA queue ramp); dummy matmuls on zeroed SBUF keep
        # the HAM clock ramping toward 2.4 GHz meanwhile.
        warm_rhs = iop.tile([P, BSH], mybir.dt.bfloat16, tag="warm_rhs", name="warm_rhs")
        nc.vector.memset(warm_rhs[:], 0.0)
        warm_ps = pp.tile([P, BSH], F32, tag="pA0", name="warm_ps")
        # ~21 warmups put the first real matmul at ~10.8us, matching the DMA
        # delivery curve (~200-390 GB/s from ~8.5us, slower on contended
        # devices) so the weight stream rarely starves the PE (a >1.5us PE
        # idle triggers a HAM down-clock to half speed; that 3.4-10us
        # straggler penalty dominates the max-core metric, so margin here
        # is worth ~0.6us of fixed cost).
        for i in range(21):
            nc.tensor.matmul(
                warm_ps[:], warm_rhs[:, 0:P], warm_rhs[:], start=True, stop=True
            )

        # All epilogues on Vector (GpSimd can't read PSUM; Scalar would need
        # the ACT-table load and is busy issuing weight DMAs). With PSUM
        # bank-group double buffering the epilogue latency never gates the
        # matmul stream, and Vector is otherwise idle (~19us work total).
        def epilogue(o, ps_ap, bias_ap, relu):
            if relu:
                nc.vector.tensor_scalar(o, ps_ap, bias_ap, 0.0, ADD, MAX)
            else:
                nc.vector.tensor_scalar_add(o, ps_ap, bias_ap)

        bank = {
            "A": ["pA0", "pA1", "pA2", "pA3"],
            "B": ["pB0", "pB1", "pB2", "pB3"],
        }

        def wave(layer, wv, kt, rhs, bias_col, out_dt, grp):
            ps = [
                pp.tile([P, BSH], F32, tag=bank[grp][i], name=f"{layer}w{wv}ps{i}")
                for i in range(WAVE_J)
            ]
            for k in range(kt):
                w, base = wslice[(layer, wv, k)]
                for j in range(WAVE_J):
                    nc.tensor.matmul(
                        ps[j][:],
                        w[:, base + j * P : base + (j + 1) * P],
                        rhs[k],
                        start=(k == 0),
                        stop=(k == kt - 1),
                    )
            outs = []
            for j in range(WAVE_J):
                o = act.tile(
                    [P, BSH], out_dt, tag=f"{layer}w{wv}o{j}", name=f"{layer}w{wv}o{j}"
                )
                epilogue(o[:], ps[j][:], bias_col(wv * WAVE_J + j), True)
                outs.append(o[:])
            return outs

        hts = []
        for wv in range(NW1):
            hts += wave("l1", wv, KT1, xts, b1c, DT, "AB"[wv % 2])
        # Filler matmuls at the fc1->fc2 boundary: the most common HAM
        # down-clock trigger is a weight-stream stall right at fc2 wave0's
        # start. Eight dummies on the bank fc2 wave0 re-zeroes anyway (its
        # previous epilogue completed during fc1 wave3, so these add no new
        # dependencies) keep the PE busy through a stall's first ~0.9us,
        # holding idle chunks below the ~1.5us down-clock threshold.
        fill = pp.tile([P, BSH], F32, tag="pA0", name="fill_fc2")
        for i in range(8):
            nc.tensor.matmul(
                fill[:], warm_rhs[:, 0:P], warm_rhs[:], start=True, stop=True
            )
        h2s = []
        for wv in range(NW2):
            h2s += wave("l2", wv, KT2, hts, b2c, DT, "AB"[wv % 2])

        # fc3 j-outer: each output tile's epilogue + store overlaps the next
        # tile's matmuls; only the last tile's epilogue+store is tail latency.
        tags8 = bank["A"] + bank["B"]
        for j in range(W3J):
            psj = pp.tile([P, BSH], F32, tag=tags8[j], name=f"l3ps{j}")
            for k in range(KT3):
                w, base = wslice[("l3", 0, k)]
                nc.tensor.matmul(
                    psj[:],
                    w[:, base + j * P : base + (j + 1) * P],
                    h2s[k],
                    start=(k == 0),
                    stop=(k == KT3 - 1),
                )
            o = act.tile([P, BSH], DT, tag=f"l3o{j}", name=f"l3o{j}")
            # Split epilogue+store across both queues: halves transfer in
            # parallel, and for the last tile the first store can issue
            # while the second epilogue half still runs — this chain is the
            # kernel's tail latency.
            h = BSH // 2
            epilogue(o[:, 0:h], psj[:, 0:h], b3c(j), False)
            nc.sync.dma_start(outT[j][:, 0:h], o[:, 0:h])
            epilogue(o[:, h:BSH], psj[:, h:BSH], b3c(j), False)
            nc.scalar.dma_start(outT[j][:, h:BSH], o[:, h:BSH])


class _LightTailTileContext(tile.TileContext):
    """TileContext with a minimal end-of-kernel sequence.

    Tile's default tail (drain + full all-engine barrier + DMA/semaphore
    reset + second barrier) costs ~8-10us on HW. For a single-TileContext
    kernel the correctness requirement at the end is just: all engines done
    and all output DMAs complete before the NEFF signals completion (the
    walrus-generated per-engine teardown follows anyway).
    """

    def _drain_and_barrier(self, tick_clock, wait_clock):
        if not hasattr(self.nc, "_tile_sem_poison_stack"):
            return super()._drain_and_barrier(tick_clock, wait_clock)
        from concourse.vector_clock import ScopedClock

        drain_inst = self.nc.sync.drain()
        wait_clock.add_sem_waits(
            drain_inst.ins, ScopedClock({None: tick_clock.global_clock})
        )
        self.nc.all_engine_barrier(sem_only=True)
        assert self.sems is not None
        popped = self.nc._tile_sem_poison_stack.pop()
        assert popped is self._sem_poison

def _build(dt_name):
    if dt_name in _CACHE:
        return _CACHE[dt_name]
    DT = {"bf16": mybir.dt.bfloat16, "f32": F32}[dt_name]

    patches = []
    if FAST_CONST:
        try:
            import concourse.bass as cbass

            # During Bass construction only, reroute GpSimd memsets (the
            # framework's const-AP init) to the much faster VectorE: they
            # gate the initial all-engine barrier.
            gps_cls = cbass.BassGpSimd

            def memset_shim(self, ap, constant):
                return self.bass.vector.memset(ap, constant)

            had = "memset" in vars(gps_cls)
            orig = vars(gps_cls).get("memset")
            gps_cls.memset = memset_shim
            patches.append((gps_cls, "memset", had, orig))
            # The barrier after const-AP init protects readers of the const
            # tiles; this kernel never reads them, so skip it.
            bar_orig = cbass.Bass.all_engine_barrier

            def bar_shim(self, *, sem_only=False):
                return None

            cbass.Bass.all_engine_barrier = bar_shim
            patches.append((cbass.Bass, "all_engine_barrier", True, bar_orig))
        except AttributeError:
            pass

    try:
        nc = bacc.Bacc(
            "TRN2",
            target_bir_lowering=False,
            debug=False,
            enable_asserts=False,
            num_devices=NCORES,
        )
    finally:
        for klass, attr, had, orig in patches:
            if had:
                setattr(klass, attr, orig)
            else:
                delattr(klass, attr)

    tc_cls = _LightTailTileContext if LIGHT_TAIL else tile.TileContext
    with tc_cls(nc) as tc:
        _emit(tc, DT)
    nc.compile()
    _CACHE[dt_name] = nc
    return nc


def _np_dt(dt_name):
    return mybir.dt.np({"bf16": mybir.dt.bfloat16, "f32": F32}[dt_name])


def _host_prep(x, W1, b1, crow_indices, col_indices, values, b2, W3, b3, npdt):
    rb = crow_indices.shape[0] - 1
    nnz, bs, _ = values.shape
    cb = H // bs
    # Scatter BSR into dense W2 [H, H].
    blocks = np.zeros((rb, cb, bs, bs), np.float32)
    row_ids = (
        np.searchsorted(crow_indices, np.arange(nnz, dtype=np.int64), side="right") - 1
    )
    blocks[row_ids, col_indices] = values
    W2 = blocks.transpose(0, 2, 1, 3).reshape(H, H)

    WT = {
        "l1": np.ascontiguousarray(W1.T).astype(npdt),  # [IN, H]
        "l2": np.ascontiguousarray(W2.T).astype(npdt),  # [H, H]
        "l3": np.ascontiguousarray(W3.T).astype(npdt),  # [H, OUT]
    }
    # Pack the streamed weight sequence: one contiguous [P, merge*width]
    # block per DMA instruction, in consumption order.
    blocks_out = []
    for e in DMA_PLAN:
        if e[0] == "x":
            continue
        _, layer, wv, k0, merge, width = e
        w = WT[layer]
        jbase = wv * width
        blocks_out.append(
            np.concatenate(
                [
                    w[(k0 + kk) * P : (k0 + kk + 1) * P, jbase : jbase + width]
                    for kk in range(merge)
                ],
                axis=1,
            )
        )
    wseq = np.ascontiguousarray(np.concatenate(blocks_out, axis=1))
    assert wseq.shape == (P, WSEQ_COLS)

    bc = np.ascontiguousarray(
        np.concatenate(
            [
                b1.reshape(W1J, P).T,
                b2.reshape(W2J, P).T,
                b3.reshape(W3J, P).T,
            ],
            axis=1,
        ).astype(np.float32)
    )
    # x -> per-core transposed shards [P, KT1, BSH] in natural k order.
    xT_all = np.ascontiguousarray(x.T.astype(npdt))  # [IN, B]
    shards = [
        np.ascontiguousarray(
            xT_all[:, c * BSH : (c + 1) * BSH].reshape(KT1, P, BSH).transpose(1, 0, 2)
        )
        for c in range(NCORES)
    ]
    shared = dict(wseq=wseq, bc=bc)
    return [dict(shared, xT=shards[c]) for c in range(NCORES)]


def kernel(x, W1, b1, crow_indices, col_indices, values, b2, W3, b3, _dt="bf16"):
    nc = _build(_dt)
    in_maps = _host_prep(
        np.asarray(x, np.float32),
        np.asarray(W1, np.float32),
        np.asarray(b1, np.float32),
        np.asarray(crow_indices),
        np.asarray(col_indices),
        np.asarray(values, np.float32),
        np.asarray(b2, np.float32),
        np.asarray(W3, np.float32),
        np.asarray(b3, np.float32),
        _np_dt(_dt),
    )
    res = bass_utils.run_bass_kernel_spmd(nc, in_maps, core_ids=list(range(NCORES)))
    out = np.concatenate(
        [res.results[c]["outT"].reshape(OUT, BSH).T for c in range(NCORES)], axis=0
    )
    return np.ascontiguousarray(out.astype(np.float32))
